# revision 1
# baseline (speedup 1.0000x reference)
"""Cost-volume kernel for Trainium2 (Bass/Tile), 8-core SPMD.

volume[n, c, d, h, w] = left[n,c,h,w] * right[n,c,h,w-d]  (0 where w < d)

Sharding: rows (flattened n,c,h = 8704) split as 1088 per core; every core
computes ALL 48 disparities for its rows. The shift is along W, so row
sharding needs no halo, and inputs are read exactly once globally
(2.3 MB/core) instead of once per core as D-sharding would require. This
minimizes HBM traffic per core (~52.4 MB: 50.1 MB output + inputs), which is
what binds: each pair of NeuronCores shares one ~716 GB/s HBM stack.

Layout: `right` is host-padded with MAX_DISP zero columns in front of every
row, so the multiply for disparity d reads the padded row at offset
MAX_DISP-d and the w < d region is zero automatically - no memsets, one
tensor_tensor per (chunk, d). Per core there are two chunks: a 1024-row main
chunk (128 partitions x 8 rows) whose loads issue first, and a 64-row tail
(64 partitions x 1 row). Every DMA is a fully contiguous DRAM region;
loads issue on the SP HWDGE ring, stores on the ACT ring. Measured: ~149-152
us/core HW exec (DVE 122 us busy, DMA ~137 us at ~380-420 GB/s/core), at the
shared-stack HBM roofline of ~2x52.5 MB / 716 GB/s per core pair.
"""

import os

import numpy as np

import concourse.bacc as bacc
import concourse.mybir as mybir
from concourse.bass_utils import run_bass_kernel_spmd
from concourse.mybir import AluOpType
from concourse.tile import TileContext

N, C, H, W = 2, 32, 136, 240
MAX_DISP = 48
NCORES = 8
R = N * C * H                   # 8704 rows total
ROWS = R // NCORES              # 1088 rows per core
PAD = MAX_DISP                  # front zero-pad columns on right
WP = W + PAD                    # 288
TAIL = 64                       # leftover rows (1088 = 64 + 128*8)
CPP = 8                         # rows per partition in the main chunk

_NC_CACHE = None
LAST_RESULTS = None  # BassKernelResults of the most recent run (for test.py)


def _build_bass():
    # Bacc (not plain Bass): its finalize() runs the compile pipeline incl.
    # generate_event_semaphores, which splits multi-sem waits that walrus
    # rejects ("Too many sync wait commands").
    nc = bacc.Bacc()
    left = nc.dram_tensor("left", [ROWS, W], mybir.dt.float32, kind="ExternalInput")
    right = nc.dram_tensor("right", [ROWS, WP], mybir.dt.float32, kind="ExternalInput")
    out = nc.dram_tensor(
        "out", [MAX_DISP, ROWS, W], mybir.dt.float32, kind="ExternalOutput"
    )

    with (
        TileContext(nc) as tc,
        tc.tile_pool(name="lpool", bufs=2) as lpool,
        tc.tile_pool(name="rpool", bufs=2) as rpool,
        tc.tile_pool(name="obig", bufs=14) as obig,
        tc.tile_pool(name="otail", bufs=14) as otail,
    ):
        # Tail chunk: rows [0, 64) as [64, W]; main chunk: rows [64, 1088)
        # as [128, 8*W] (8 consecutive rows per partition).
        lt = lpool.tile([TAIL, W], mybir.dt.float32, tag="ltail")
        rt = rpool.tile([TAIL, WP], mybir.dt.float32, tag="rtail")
        lb = lpool.tile([128, CPP * W], mybir.dt.float32, tag="lbig")
        rb = rpool.tile([128, CPP * WP], mybir.dt.float32, tag="rbig")
        # Big loads first: they carry ~2.3MB of engine work, so issuing them
        # at the earliest slots keeps the SDMA engines fed through the ramp.
        nc.sync.dma_start(
            out=lb[:],
            in_=left[TAIL:ROWS, :].rearrange("(p q) w -> p (q w)", p=128),
        )
        nc.sync.dma_start(
            out=rb[:],
            in_=right[TAIL:ROWS, :].rearrange("(p q) w -> p (q w)", p=128),
        )
        nc.sync.dma_start(out=lt[:], in_=left[0:TAIL, :])
        nc.sync.dma_start(out=rt[:], in_=right[0:TAIL, :])
        lbview = lb[:].rearrange("p (q w) -> p q w", w=W)
        rbview = rb[:].rearrange("p (q w) -> p q w", w=WP)
        for d in range(MAX_DISP):
            ob = obig.tile([128, CPP * W], mybir.dt.float32)
            nc.vector.tensor_tensor(
                ob[:].rearrange("p (q w) -> p q w", w=W),
                lbview,
                rbview[:, :, PAD - d : PAD - d + W],
                AluOpType.mult,
            )
            nc.scalar.dma_start(
                out=out[d, TAIL:ROWS, :].rearrange("(p q) w -> p (q w)", p=128),
                in_=ob[:],
            )
            ot = otail.tile([TAIL, W], mybir.dt.float32)
            nc.vector.tensor_tensor(
                ot[:], lt[:], rt[:, PAD - d : PAD - d + W], AluOpType.mult
            )
            nc.scalar.dma_start(out=out[d, 0:TAIL, :], in_=ot[:])
    nc.finalize()
    return nc


def kernel(left: np.ndarray, right: np.ndarray) -> np.ndarray:
    global _NC_CACHE, LAST_RESULTS
    left = np.ascontiguousarray(np.asarray(left, dtype=np.float32))
    right = np.ascontiguousarray(np.asarray(right, dtype=np.float32))
    assert left.shape == (N, C, H, W) and right.shape == (N, C, H, W)

    if _NC_CACHE is None:
        _NC_CACHE = _build_bass()
    nc = _NC_CACHE

    left_flat = left.reshape(R, W)
    right_pad = np.zeros((R, WP), dtype=np.float32)
    right_pad[:, PAD:] = right.reshape(R, W)
    in_maps = [
        {
            "left": left_flat[ROWS * k : ROWS * (k + 1)],
            "right": right_pad[ROWS * k : ROWS * (k + 1)],
        }
        for k in range(NCORES)
    ]

    trace = os.environ.get("COSTVOL_TRACE", "0") == "1"
    kwargs = {}
    if os.environ.get("COSTVOL_TRACE_ALL", "0") == "1":
        kwargs["trace_cores"] = list(range(NCORES))
    res = run_bass_kernel_spmd(
        nc, in_maps, list(range(NCORES)), trace=trace, **kwargs
    )
    LAST_RESULTS = res

    flat = np.empty((MAX_DISP, R, W), dtype=np.float32)
    for k in range(NCORES):
        flat[:, ROWS * k : ROWS * (k + 1), :] = res.results[k]["out"]
    vol = flat.reshape(MAX_DISP, N, C, H, W).transpose(1, 2, 0, 3, 4)
    return np.ascontiguousarray(vol)

